# revision 20
# baseline (speedup 1.0000x reference)
"""Trainium2 Bass kernel for nn_HashDecoder (multiresolution hash encoding + MLP).

Strategy: data-parallel over 8 NeuronCores — each core gets N/8 points and a
replicated hash table. Per core: hash indices computed on DVE (exact integer
math via 13-bit prime splits under the fp32-backed int ALU), per-corner table
rows fetched from HBM with SWDGE indirect DMAs (128 rows/instruction),
trilinear weighted sum on DVE, and the 3-layer MLP on the PE via per-quadrant
transposes.  Self-contained: hardcodes shapes/sharding for
p=[1048576,3], hash_table=[8388608,2], w1/w2/w3.
"""
import numpy as np

import concourse.bass as bass
from concourse import bacc, mybir
from concourse.tile import TileContext
from concourse.masks import make_identity
from concourse.bass import ds

F32 = mybir.dt.float32
I32 = mybir.dt.int32
A = mybir.AluOpType
AF = mybir.ActivationFunctionType

NUM_LEVELS = 16
V = 1 << 19
MASK = V - 1
P1L = 2654435761 & MASK
P2L = 805459861 & MASK
P1_LO, P1_HI = P1L & 0x1FFF, P1L >> 13
P2_LO, P2_HI = P2L & 0x1FFF, P2L >> 13
# corner order (nerfstudio); 1 => ceil coord
CORNERS = [(1,1,1),(1,0,1),(0,0,1),(0,1,1),(1,1,0),(1,0,0),(0,0,0),(0,1,0)]


def scalings():
    growth = np.exp((np.log(1024.) - np.log(16.)) / (NUM_LEVELS - 1))
    return np.floor(16. * growth ** np.arange(NUM_LEVELS)).astype(np.float32)


def build(N_core, F=256, B=8, UNROLL=4, n_levels=NUM_LEVELS):
    """Build the per-core Bass program. Points laid out n = tile*128*F + part*F + col."""
    T = N_core // (128 * F)
    assert T * 128 * F == N_core

    nc = bacc.Bacc("TRN2", target_bir_lowering=False, debug=False, num_devices=8)
    pt_d = nc.dram_tensor("pt", [3, N_core], F32, kind="ExternalInput")
    tb_d = nc.dram_tensor("table", [NUM_LEVELS * V, 2], F32, kind="ExternalInput")
    w1_d = nc.dram_tensor("w1", [32, 32], F32, kind="ExternalInput")
    w2_d = nc.dram_tensor("w2", [32, 32], F32, kind="ExternalInput")
    w3_d = nc.dram_tensor("w3", [32, 4], F32, kind="ExternalInput")
    cst_d = nc.dram_tensor("consts", [128, 2 * NUM_LEVELS], F32, kind="ExternalInput")
    # consts[:, 0:16] = scales (f32); consts[:, 16:32] = lvl offsets bitcast from int32
    out_d = nc.dram_tensor("out", [N_core, 4], F32, kind="ExternalOutput")

    ts, tt = nc.vector.tensor_scalar, nc.vector.tensor_tensor

    with TileContext(nc) as tc:
        with tc.tile_pool(name="pm", bufs=1) as pm, \
             tc.tile_pool(name="lvp", bufs=1) as lvp, \
             tc.tile_pool(name="gp", bufs=1) as gp, \
             tc.tile_pool(name="st", bufs=4) as st, \
             tc.tile_pool(name="mst", bufs=2) as mst, \
             tc.tile_pool(name="ps", bufs=1, space="PSUM") as ps:

            ident = pm.tile([128, 128], F32, tag="ident")
            make_identity(nc, ident[:])
            ident4 = pm.tile([4, 4], F32, tag="ident4")
            make_identity(nc, ident4[:])
            w1t = pm.tile([128, 32], F32, tag="w1t")
            w2t = pm.tile([128, 32], F32, tag="w2t")
            w3t = pm.tile([128, 4], F32, tag="w3t")
            for q in range(4):
                nc.sync.dma_start(out=w1t[32*q:32*q+32, :], in_=w1_d.ap()[:])
                nc.sync.dma_start(out=w2t[32*q:32*q+32, :], in_=w2_d.ap()[:])
                nc.sync.dma_start(out=w3t[32*q:32*q+32, :], in_=w3_d.ap()[:])
            cst = pm.tile([128, 2 * NUM_LEVELS], F32, tag="cst")
            nc.sync.dma_start(out=cst[:], in_=cst_d.ap()[:])
            scal_ap = cst[:, 0:NUM_LEVELS]
            lvo_ap = cst[:, NUM_LEVELS:2 * NUM_LEVELS].bitcast(I32)

            for t in range(T):
                n0 = t * 128 * F
                px = pm.tile([128, F], F32, tag="px")
                py = pm.tile([128, F], F32, tag="py")
                pz = pm.tile([128, F], F32, tag="pz")
                nc.sync.dma_start(out=px[:], in_=pt_d.ap()[0, n0:n0 + 128 * F].rearrange("(p f) -> p f", p=128))
                nc.sync.dma_start(out=py[:], in_=pt_d.ap()[1, n0:n0 + 128 * F].rearrange("(p f) -> p f", p=128))
                nc.sync.dma_start(out=pz[:], in_=pt_d.ap()[2, n0:n0 + 128 * F].rearrange("(p f) -> p f", p=128))
                enc = pm.tile([128, F, 2 * NUM_LEVELS], F32, tag="enc")
                out_tile = pm.tile([128, F, 4], F32, tag="out_tile")

                def lv_body(lv):
                    sc = scal_ap[:, ds(lv, 1)]
                    lvo = lvo_ap[:, ds(lv, 1)]

                    def coord(pf, tag):
                        s = lvp.tile([128, F], F32, tag=f"s{tag}")
                        ts(out=s[:], in0=pf[:], scalar1=sc, scalar2=None, op0=A.mult)
                        sm = lvp.tile([128, F], F32, tag=f"sm{tag}")
                        ts(out=sm[:], in0=s[:], scalar1=-0.5, scalar2=None, op0=A.add)
                        ci = lvp.tile([128, F], I32, tag=f"ci{tag}")
                        nc.vector.tensor_copy(out=ci[:], in_=sm[:])
                        cf = lvp.tile([128, F], F32, tag=f"cf{tag}")
                        nc.vector.tensor_copy(out=cf[:], in_=ci[:])
                        off = lvp.tile([128, F], F32, tag=f"off{tag}")
                        tt(out=off[:], in0=s[:], in1=cf[:], op=A.subtract)
                        return ci, cf, off

                    xi, _, ox = coord(px, "x")
                    yi, yf, oy = coord(py, "y")
                    zi, zf, oz = coord(pz, "z")

                    def hpair(cf_, lo, hi, padd, tag, add_lvo):
                        t1 = lvp.tile([128, F], F32, tag=f"hp{tag}")
                        ts(out=t1[:], in0=cf_[:], scalar1=float(lo), scalar2=None, op0=A.mult)
                        i1 = lvp.tile([128, F], I32, tag=f"hpi{tag}")
                        nc.vector.tensor_copy(out=i1[:], in_=t1[:])
                        ts(out=t1[:], in0=cf_[:], scalar1=float(hi), scalar2=None, op0=A.mult)
                        i2 = lvp.tile([128, F], I32, tag=f"hpj{tag}")
                        nc.vector.tensor_copy(out=i2[:], in_=t1[:])
                        ts(out=i2[:], in0=i2[:], scalar1=63, scalar2=None, op0=A.bitwise_and)
                        ts(out=i2[:], in0=i2[:], scalar1=8192, scalar2=None, op0=A.mult)
                        a0 = lvp.tile([128, F], I32, tag=f"a0{tag}")
                        tt(out=a0[:], in0=i1[:], in1=i2[:], op=A.add)
                        ts(out=a0[:], in0=a0[:], scalar1=MASK, scalar2=None, op0=A.bitwise_and)
                        a1 = lvp.tile([128, F], I32, tag=f"a1{tag}")
                        ts(out=a1[:], in0=a0[:], scalar1=padd, scalar2=None, op0=A.add)
                        ts(out=a1[:], in0=a1[:], scalar1=MASK, scalar2=None, op0=A.bitwise_and)
                        if add_lvo:
                            tt(out=a0[:], in0=a0[:], in1=lvo.to_broadcast([128, F]), op=A.add)
                            tt(out=a1[:], in0=a1[:], in1=lvo.to_broadcast([128, F]), op=A.add)
                        return a0, a1

                    ay0, ay1 = hpair(yf, P1_LO, P1_HI, P1L, "y", False)
                    az0, az1 = hpair(zf, P2_LO, P2_HI, P2L, "z", True)

                    t_ = {}
                    for a_, ya in ((0, ay0), (1, ay1)):
                        for b_, za in ((0, az0), (1, az1)):
                            tl = lvp.tile([128, F], I32, tag=f"t{a_}{b_}")
                            tt(out=tl[:], in0=ya[:], in1=za[:], op=A.bitwise_xor)
                            t_[(a_, b_)] = tl
                    xi1 = lvp.tile([128, F], I32, tag="xi1")
                    ts(out=xi1[:], in0=xi[:], scalar1=1, scalar2=None, op0=A.add)

                    hsup = lvp.tile([128, F, 8], I32, tag="hsup")
                    for c, (mx, my, mz) in enumerate(CORNERS):
                        tt(out=hsup[:, :, c], in0=(xi1 if mx else xi)[:], in1=t_[(my, mz)][:], op=A.bitwise_xor)

                    # weights
                    wx0 = lvp.tile([128, F], F32, tag="wx0"); ts(out=wx0[:], in0=ox[:], scalar1=-1.0, scalar2=1.0, op0=A.mult, op1=A.add)
                    wy0 = lvp.tile([128, F], F32, tag="wy0"); ts(out=wy0[:], in0=oy[:], scalar1=-1.0, scalar2=1.0, op0=A.mult, op1=A.add)
                    wz0 = lvp.tile([128, F], F32, tag="wz0"); ts(out=wz0[:], in0=oz[:], scalar1=-1.0, scalar2=1.0, op0=A.mult, op1=A.add)
                    wyz = {}
                    for a_, ya in ((0, wy0), (1, oy)):
                        for b_, za in ((0, wz0), (1, oz)):
                            w = lvp.tile([128, F], F32, tag=f"wyz{a_}{b_}")
                            tt(out=w[:], in0=ya[:], in1=za[:], op=A.mult)
                            wyz[(a_, b_)] = w
                    wsup = lvp.tile([128, F, 8], F32, tag="wsup")
                    for c, (mx, my, mz) in enumerate(CORNERS):
                        tt(out=wsup[:, :, c], in0=(ox if mx else wx0)[:], in1=wyz[(my, mz)][:], op=A.mult)

                    # gather loop
                    gsup = gp.tile([128, F, 2, 8], F32, tag="gsup")

                    def blk_body(blk):
                        si = st.tile([128, B * 8], I32, tag="si")
                        nc.vector.tensor_copy(out=si[:], in_=hsup[:, ds(blk * B, B), :].rearrange("p b c -> p (b c)"))
                        sg = st.tile([128, B * 8, 2], F32, tag="sg")
                        for k in range(B * 8):
                            nc.gpsimd.indirect_dma_start(
                                out=sg[:, k, :], out_offset=None, in_=tb_d.ap(),
                                in_offset=bass.IndirectOffsetOnAxis(ap=si[:, k:k+1], axis=0),
                            )
                        nc.vector.tensor_copy(
                            out=gsup[:, ds(blk * B, B), :, :].rearrange("p b f c -> p b c f"),
                            in_=sg[:].rearrange("p (b c) f -> p b c f", b=B),
                        )
                    tc.For_i_unrolled(0, F // B, 1, blk_body, max_unroll=UNROLL)

                    # weighted sum: prod = gsup * wsup; enc[..., 2lv:2lv+2] = sum over corners
                    prod = gp.tile([128, F, 2, 8], F32, tag="prod")
                    tt(out=prod[:], in0=gsup[:],
                       in1=wsup[:].unsqueeze(2).to_broadcast([128, F, 2, 8]),
                       op=A.mult)
                    nc.vector.tensor_reduce(
                        enc[:, :, ds(lv * 2, 2)].unsqueeze(3),
                        prod[:], mybir.AxisListType.X, A.add)

                with tc.For_i(0, n_levels, 1) as lv:
                    lv_body(lv)

                # ---- MLP ----
                def mlp_body(fq):
                    trbig = mst.tile([128, 512], F32, tag="trbig")
                    for t4 in range(4):
                        se = mst.tile([128, 128], F32, tag="se")
                        nc.vector.tensor_copy(out=se[:], in_=enc[:, ds(fq * 16 + t4 * 4, 4), :].rearrange("p a b -> p (a b)"))
                        pst = ps.tile([128, 128], F32, tag="pst", space="PSUM")
                        nc.tensor.transpose(out=pst[:], in_=se[:], identity=ident[:])
                        nc.vector.tensor_copy(out=trbig[:, t4 * 128:(t4 + 1) * 128], in_=pst[:])
                    for c4 in range(4):
                        rhs = trbig[32 * c4:32 * c4 + 32, :]
                        ps1 = ps.tile([32, 512], F32, tag="ps1", space="PSUM")
                        nc.tensor.matmul(ps1[:], w1t[32*c4:32*c4+32, :], rhs, start=True, stop=True, tile_position=(32*c4, 0))
                        s1 = mst.tile([32, 512], F32, tag="s1")
                        nc.scalar.activation(out=s1[:], in_=ps1[:], func=AF.Relu)
                        ps2 = ps.tile([32, 512], F32, tag="ps2", space="PSUM")
                        nc.tensor.matmul(ps2[:], w2t[0:32, :], s1[:], start=True, stop=True)
                        s2 = mst.tile([32, 512], F32, tag="s2")
                        nc.scalar.activation(out=s2[:], in_=ps2[:], func=AF.Relu)
                        ps3 = ps.tile([4, 512], F32, tag="ps3", space="PSUM")
                        nc.tensor.matmul(ps3[:], w3t[0:32, :], s2[:], start=True, stop=True)
                        s3 = mst.tile([4, 512], F32, tag="s3")
                        nc.vector.tensor_copy(out=s3[:], in_=ps3[:])
                        for t4 in range(4):
                            ptb = ps.tile([128, 4], F32, tag="ptb", space="PSUM")
                            nc.tensor.transpose(out=ptb[:], in_=s3[:, 128 * t4:128 * (t4 + 1)], identity=ident4[:])
                            nc.vector.tensor_copy(out=out_tile[:, ds(fq * 16 + t4 * 4 + c4, 1), :].rearrange("p a b -> p (a b)"), in_=ptb[:])
                with tc.For_i(0, F // 16, 1) as fq:
                    mlp_body(fq)

                nc.sync.dma_start(
                    out=out_d.ap()[n0:n0 + 128 * F, :].rearrange("(p f) o -> p f o", p=128),
                    in_=out_tile[:])
    nc.compile()
    return nc


def make_consts():
    cst = np.zeros((128, 2 * NUM_LEVELS), dtype=np.float32)
    cst[:, :NUM_LEVELS] = scalings()[None, :]
    cst[:, NUM_LEVELS:] = (np.arange(NUM_LEVELS, dtype=np.int32) * V)[None, :].view(np.float32)
    return cst


_N = 1 << 20
_NCORES = 8


_PROG = {}


def _get_prog():
    if "nc" not in _PROG:
        _PROG["nc"] = build(_N // _NCORES, F=256, B=8, UNROLL=4)
    return _PROG["nc"]


def _in_maps(p, hash_table, w1, w2, w3):
    N_core = _N // _NCORES
    consts = make_consts()
    table = np.ascontiguousarray(hash_table).astype(np.float32)
    maps = []
    for c in range(_NCORES):
        sl = np.asarray(p[c * N_core:(c + 1) * N_core])
        maps.append({
            "pt": np.ascontiguousarray(sl.T).astype(np.float32),
            "table": table,
            "w1": np.ascontiguousarray(w1).astype(np.float32),
            "w2": np.ascontiguousarray(w2).astype(np.float32),
            "w3": np.ascontiguousarray(w3).astype(np.float32),
            "consts": consts,
        })
    return maps


def kernel(p, hash_table, w1, w2, w3):
    nc = _get_prog()
    from concourse.bass_utils import run_bass_kernel_spmd
    in_maps = _in_maps(p, hash_table, w1, w2, w3)
    res = run_bass_kernel_spmd(nc, in_maps, core_ids=list(range(_NCORES)))
    out = np.concatenate([res.results[c]["out"] for c in range(_NCORES)], axis=0)
    return out.astype(np.float32)


# ---------------------------------------------------------------------------
# HW timing: the axon client in this container has no NTFF profiling hook, so
# measure steady-state wall-clock of K back-to-back dispatches of the NEFF
# (queued async, blocked once at the end) vs 1 dispatch, and report
# (t_K - t_1) / (K - 1).  Fixed dispatch/tunnel overhead cancels in the
# difference; what remains is the on-device execution time.
# ---------------------------------------------------------------------------

def _build_timed_fn(nc, n_cores):
    import jax
    from jax.sharding import Mesh, PartitionSpec
    from jax.experimental.shard_map import shard_map
    from concourse import bass2jax, mybir as mb
    bass2jax.install_neuronx_cc_hook()

    partition_name = nc.partition_id_tensor.name if nc.partition_id_tensor else None
    in_names, out_names, out_avals = [], [], []
    for alloc in nc.m.functions[0].allocations:
        if not isinstance(alloc, mb.MemoryLocationSet):
            continue
        name = alloc.memorylocations[0].name
        if alloc.kind == "ExternalInput":
            if name != partition_name:
                in_names.append(name)
        elif alloc.kind == "ExternalOutput":
            out_names.append(name)
            out_avals.append(jax.core.ShapedArray(tuple(alloc.tensor_shape), mb.dt.np(alloc.dtype)))
    n_params = len(in_names)
    n_outs = len(out_names)
    bind_in_names = tuple(in_names + out_names + ([partition_name] if partition_name else []))

    def _body(*args):
        operands = list(args)
        if partition_name is not None:
            operands.append(bass2jax.partition_id_tensor())
        return tuple(bass2jax._bass_exec_p.bind(
            *operands,
            out_avals=tuple(out_avals),
            in_names=bind_in_names,
            out_names=tuple(out_names),
            lowering_input_output_aliases=(),
            sim_require_finite=True,
            sim_require_nnan=True,
            nc=nc,
        ))

    donate = tuple(range(n_params, n_params + n_outs))
    devices = jax.devices()[:n_cores]
    mesh = Mesh(np.asarray(devices), ("core",))
    in_specs = (PartitionSpec("core"),) * (n_params + n_outs)
    out_specs = (PartitionSpec("core"),) * n_outs
    fn = jax.jit(
        shard_map(_body, mesh=mesh, in_specs=in_specs, out_specs=out_specs, check_rep=False),
        donate_argnums=donate, keep_unused=True,
    )
    return fn, in_names, out_names, out_avals, mesh


def timed_run(inputs, tmpdir=None, n_reps=4, n_timing_iters=3):
    """Measure per-execution HW time via the K-vs-1 dispatch wall-clock delta."""
    import time as _time
    import jax
    from jax.sharding import NamedSharding, PartitionSpec

    nc = _get_prog()
    in_maps = _in_maps(inputs["p"], inputs["hash_table"], inputs["w1"],
                       inputs["w2"], inputs["w3"])
    n_cores = _NCORES

    fn, in_names, out_names, out_avals, mesh = _build_timed_fn(nc, n_cores)
    sharding = NamedSharding(mesh, PartitionSpec("core"))
    concat_in = [
        jax.device_put(
            np.concatenate([np.asarray(in_maps[c][name]) for c in range(n_cores)], axis=0),
            sharding)
        for name in in_names
    ]

    def zero_set():
        return [
            jax.device_put(
                np.zeros((n_cores * av.shape[0], *av.shape[1:]), av.dtype), sharding)
            for av in out_avals
        ]

    out = fn(*concat_in, *zero_set())  # warmup (compile + first exec)
    jax.block_until_ready(out)

    def run_k(k):
        zsets = [zero_set() for _ in range(k)]
        jax.block_until_ready(zsets)
        t0 = _time.perf_counter()
        outs = [fn(*concat_in, *z) for z in zsets]
        jax.block_until_ready(outs)
        return _time.perf_counter() - t0

    t1 = min(run_k(1) for _ in range(n_timing_iters))
    tk = min(run_k(n_reps) for _ in range(n_timing_iters))
    exec_s = (tk - t1) / (n_reps - 1)
    print(f"[timed_run] t1={t1*1e3:.2f} ms  t{n_reps}={tk*1e3:.2f} ms  "
          f"-> per-exec {exec_s*1e3:.3f} ms")
    return max(0, int(round(exec_s * 1e9)))
